# revision 7
# baseline (speedup 1.0000x reference)
"""Trainium2 Bass kernel for debiased Sinkhorn divergence loss (geomloss-style).

Problem: B=8 batch of point clouds x,y [1024, 3]; loss = mean_b(
  (OT(x,y) - 0.5*OT(x,x) - 0.5*OT(y,y)) / N ), each OT via 17-step
log-domain Sinkhorn with geometric epsilon annealing.

Sharding: data-parallel over batch — each of the 8 NeuronCores runs one
batch element's three Sinkhorn problems; host combines the 24 OT values.

Device algorithm (per core), absorption form (validated == reference to
~1e-7 rel):
  g_new = g - eps*log( sum_i exp( (f_i + g_j - C_ij)/eps + log(1/N) ) )
  f_new = f - eps*log( sum_j exp( (g_j + f_i - C_ij)/eps + log(1/N) ) )
Cost matrices C (and C^T for the xy pair) are built on the PE from
host-prepared rank-5 factors.  Reductions always run along the SBUF free
dim: the per-partition potential enters as the ACT bias, the free-dim
potential is broadcast via PE rank-1 matmuls, the C term is fused in a
single DVE scalar_tensor_tensor pass, and exp+row-sum is one ACT pass
(accum_out).  The eps schedule is data-dependent (max over the batch of
each C stack) and is computed on host, entering as tiny input tables.
"""

import sys
import numpy as np

for _p in ("/opt/trn_rl_repo", "/root/.axon_site/_ro/trn_rl_repo"):
    if _p not in sys.path:
        sys.path.insert(0, _p)

_N = 1024          # points per cloud
_NT = 8            # 128-row tiles per matrix
_B = 8             # batch == cores
_NITER = 17        # 12 anneal + 5 extra
_EPS_FINAL = np.float32(0.05) ** np.float32(2.0)
_LOG_INV_N = float(-np.log(np.float32(_N)))

_cached = {}


def _build_program():
    import concourse.bass as bass
    import concourse.mybir as mybir
    from concourse import bacc, tile

    F32 = mybir.dt.float32
    AO = mybir.AluOpType
    AF = mybir.ActivationFunctionType

    nc = bacc.Bacc("TRN2", target_bir_lowering=False, debug=False,
                   enable_asserts=False)

    def din(name, shape):
        return nc.dram_tensor(name, shape, F32, kind="ExternalInput").ap()

    # rank-5 cost factors: L* = [x0,x1,x2, 0.5*|x|^2, 1], R* = [-x0,-x1,-x2, 1, 0.5*|x|^2]
    Lx = din("Lx", [5, _N])
    Ly = din("Ly", [5, _N])
    Rx = din("Rx", [5, _N])
    Ry = din("Ry", [5, _N])
    ie = din("ie", [128, 3 * _NITER])    # 1/eps   per (grp,iter), col g*17+t
    nie = din("nie", [128, 3 * _NITER])  # -1/eps
    nep = din("nep", [128, 3 * _NITER])  # -eps
    ident = din("ident", [128, 128])     # identity for PE transpose
    sel = din("sel", [8, _N])            # sel[k, 128u+p] = (k==u), rank-1 selectors
    out_d = nc.dram_tensor("out", [6, 128, _NT], F32, kind="ExternalOutput").ap()

    with tile.TileContext(nc) as tc:
        with (
            tc.tile_pool(name="cm", bufs=1) as cm_pool,
            tc.tile_pool(name="const", bufs=1) as const_pool,
            tc.tile_pool(name="state", bufs=2) as st_pool,
            tc.tile_pool(name="small", bufs=3) as sm_pool,
            tc.tile_pool(name="arg", bufs=3) as arg_pool,
            tc.tile_pool(name="escr", bufs=2) as e_pool,
        ):
            # ---- constants ----
            sel_sb = const_pool.tile([8, _N], F32, tag="sel")
            nc.sync.dma_start(sel_sb[:], sel[:])
            ident_sb = const_pool.tile([128, 128], F32, tag="ident")
            nc.sync.dma_start(ident_sb[:], ident[:])
            ie_sb = const_pool.tile([128, 3 * _NITER], F32, tag="ie")
            nie_sb = const_pool.tile([128, 3 * _NITER], F32, tag="nie")
            nep_sb = const_pool.tile([128, 3 * _NITER], F32, tag="nep")
            nc.sync.dma_start(ie_sb[:], ie[:])
            nc.sync.dma_start(nie_sb[:], nie[:])
            nc.sync.dma_start(nep_sb[:], nep[:])

            # ---- cost matrices ----
            # grp 0: xy needs C [i,j] and CT [j,i]; grp 1 (xx) / 2 (yy) are
            # symmetric so one matrix serves both update directions.
            with tc.tile_pool(name="fac", bufs=1) as fac_pool, \
                 tc.tile_pool(name="psC", bufs=4, space=bass.MemorySpace.PSUM) as ps_setup:
                facs = {}
                for nm, dr in (("Lx", Lx), ("Ly", Ly), ("Rx", Rx), ("Ry", Ry)):
                    ft = fac_pool.tile([5, _N], F32, tag=nm)
                    nc.sync.dma_start(ft[:], dr[:])
                    facs[nm] = ft

                cmats = {}
                specs = [("Cxy", "Lx", "Ry"), ("CTxy", "Ly", "Rx"),
                         ("Cxx", "Lx", "Rx"), ("Cyy", "Ly", "Ry")]
                k = 0
                for cname, lf, rf in specs:
                    ct = cm_pool.tile([128, _NT * _N], F32, tag=cname)
                    cmats[cname] = ct
                    for u in range(_NT):
                        for h in range(2):
                            ps = ps_setup.tile([128, 512], F32, tag="psC")
                            nc.tensor.matmul(
                                ps[:],
                                lhsT=facs[lf][:, u * 128:(u + 1) * 128],
                                rhs=facs[rf][:, h * 512:(h + 1) * 512],
                                start=True, stop=True,
                            )
                            dst = ct[:, u * _N + h * 512: u * _N + (h + 1) * 512]
                            if k % 2 == 0:
                                nc.vector.tensor_copy(dst, ps[:])
                            else:
                                nc.scalar.copy(dst, ps[:])
                            k += 1

            # matrices used by (g-update, f-update) per group
            mat_g = [cmats["CTxy"], cmats["Cxx"], cmats["Cyy"]]
            mat_f = [cmats["Cxy"], cmats["Cxx"], cmats["Cyy"]]

            with (
                tc.tile_pool(name="psR", bufs=3, space=bass.MemorySpace.PSUM) as ps_rank1,
                tc.tile_pool(name="psT", bufs=2, space=bass.MemorySpace.PSUM) as ps_tpose,
            ):
                # ---- initial potentials (zero) ----
                fcols = []
                gcols = []
                for g in range(3):
                    fz = st_pool.tile([128, _NT], F32, tag=f"fc{g}")
                    gz = st_pool.tile([128, _NT], F32, tag=f"gc{g}")
                    nc.vector.memset(fz[:], 0.0)
                    nc.vector.memset(gz[:], 0.0)
                    fcols.append(fz)
                    gcols.append(gz)

                def half_update(grp, t, cmat, bias_cols, bcast_cols, new_tag):
                    """One Sinkhorn half-step. Returns the new potential cols.

                    bias_cols: the potential being updated (enters ACT bias).
                    bcast_cols: the other potential (broadcast along free dim).
                    """
                    idx = grp * _NITER + t
                    # scale the broadcast-side potential by 1/eps, transpose
                    sc = sm_pool.tile([128, _NT], F32, tag="sc")
                    nc.vector.tensor_scalar(
                        out=sc[:], in0=bcast_cols[:],
                        scalar1=ie_sb[:, idx:idx + 1], scalar2=None, op0=AO.mult)
                    tp = ps_tpose.tile([_NT, 128], F32, tag="tp")
                    nc.tensor.transpose(tp[:], sc[:], ident_sb[:])
                    rowv = sm_pool.tile([_NT, 128], F32, tag="rowv")
                    nc.vector.tensor_copy(rowv[:], tp[:])
                    # broadcast to [128, N] via selector matmuls:
                    # out[p, 128u+q] = sum_k sel[k,128u+p]? no — sel block u is
                    # a row of ones at partition u, so sel_u.T @ rowv = rowv[u]
                    # replicated on all 128 partitions.
                    r1 = ps_rank1.tile([128, _N], F32, tag="r1")
                    for u in range(_NT):
                        nc.tensor.matmul(
                            r1[:, u * 128:(u + 1) * 128],
                            lhsT=sel_sb[:, u * 128:(u + 1) * 128],
                            rhs=rowv[:, :],
                            start=True, stop=True)
                    # ACT bias: bias_cols/eps + log(1/N)
                    bias = sm_pool.tile([128, _NT], F32, tag="bias")
                    nc.vector.tensor_scalar(
                        out=bias[:], in0=bias_cols[:],
                        scalar1=ie_sb[:, idx:idx + 1], scalar2=_LOG_INV_N,
                        op0=AO.mult, op1=AO.add)
                    S = sm_pool.tile([128, _NT], F32, tag="S")
                    for u in range(_NT):
                        argt = arg_pool.tile([128, _N], F32, tag="arg")
                        nc.vector.scalar_tensor_tensor(
                            out=argt[:],
                            in0=cmat[:, u * _N:(u + 1) * _N],
                            scalar=nie_sb[:, idx:idx + 1],
                            in1=r1[:],
                            op0=AO.mult, op1=AO.add)
                        et = e_pool.tile([128, _N], F32, tag="E")
                        nc.scalar.activation(
                            et[:], argt[:], AF.Exp,
                            bias=bias[:, u:u + 1], scale=1.0,
                            accum_out=S[:, u:u + 1])
                    logS = sm_pool.tile([128, _NT], F32, tag="logS")
                    nc.scalar.activation(logS[:], S[:], AF.Ln)
                    new_cols = st_pool.tile([128, _NT], F32, tag=new_tag)
                    nc.vector.scalar_tensor_tensor(
                        out=new_cols[:], in0=logS[:],
                        scalar=nep_sb[:, idx:idx + 1], in1=bias_cols[:],
                        op0=AO.mult, op1=AO.add)
                    return new_cols

                for t in range(_NITER):
                    for g in range(3):
                        gcols[g] = half_update(
                            g, t, mat_g[g], gcols[g], fcols[g], f"gc{g}")
                        fcols[g] = half_update(
                            g, t, mat_f[g], fcols[g], gcols[g], f"fc{g}")

                for g in range(3):
                    nc.sync.dma_start(out_d[2 * g], fcols[g][:, :])
                    nc.sync.dma_start(out_d[2 * g + 1], gcols[g][:, :])

    nc.compile()
    return nc


def _get_program():
    if "nc" not in _cached:
        _cached["nc"] = _build_program()
    return _cached["nc"]


def _host_prep(template, source):
    """Per-core input tensors + shared eps tables (computed from batch max)."""
    template = np.asarray(template, np.float32)
    source = np.asarray(source, np.float32)
    onev = np.ones(_N, np.float32)

    def lfac(x):
        x2 = (x * x).sum(-1).astype(np.float32)
        return np.ascontiguousarray(
            np.stack([x[:, 0], x[:, 1], x[:, 2],
                      np.float32(0.5) * x2, onev]))

    def rfac(x):
        x2 = (x * x).sum(-1).astype(np.float32)
        return np.ascontiguousarray(
            np.stack([-x[:, 0], -x[:, 1], -x[:, 2],
                      onev, np.float32(0.5) * x2]))

    def cost_max(x, y):
        # fp32 like the reference; only the batch max is consumed
        x2 = (x * x).sum(-1)
        y2 = (y * y).sum(-1)
        xy = np.einsum("bnd,bmd->bnm", x, y, dtype=np.float32)
        c = np.float32(0.5) * (x2[:, :, None] + y2[:, None, :] - 2.0 * xy)
        return np.float32(c.max())

    scheds = []
    for cmax in (cost_max(template, source),
                 cost_max(template, template),
                 cost_max(source, source)):
        eps_start = np.maximum(cmax, np.float32(2.0) * _EPS_FINAL)
        t = np.arange(12, dtype=np.float32) / np.float32(11.0)
        sch = (eps_start * (_EPS_FINAL / eps_start) ** t).astype(np.float32)
        scheds.append(np.concatenate(
            [sch, np.full(5, _EPS_FINAL, np.float32)]))
    eps = np.concatenate(scheds)                       # [51]
    ie = np.broadcast_to(np.float32(1.0) / eps, (128, 51)).copy()
    nie = np.broadcast_to(np.float32(-1.0) / eps, (128, 51)).copy()
    nep = np.broadcast_to(-eps, (128, 51)).copy()
    ident = np.eye(128, dtype=np.float32)
    sel = np.repeat(np.eye(8, dtype=np.float32), 128, axis=1)

    in_maps = []
    for b in range(_B):
        x, y = template[b], source[b]
        in_maps.append({
            "Lx": lfac(x), "Ly": lfac(y),
            "Rx": rfac(x), "Ry": rfac(y),
            "ie": ie, "nie": nie, "nep": nep, "ident": ident, "sel": sel,
        })
    return in_maps, eps


def _combine(results):
    """results: per-core dict with 'out' [6,128,8] -> scalar loss."""
    ots = np.zeros((3, _B), np.float32)
    for b, res in enumerate(results):
        o = np.asarray(res["out"], np.float32)
        for g in range(3):
            ots[g, b] = o[2 * g].mean(dtype=np.float32) + \
                o[2 * g + 1].mean(dtype=np.float32)
    div = ots[0] - np.float32(0.5) * (ots[1] + ots[2])
    return np.float32((div / np.float32(_N)).mean(dtype=np.float32))


def kernel(template, source):
    from concourse.bass_utils import run_bass_kernel_spmd

    nc = _get_program()
    in_maps, _ = _host_prep(template, source)
    res = run_bass_kernel_spmd(nc, in_maps, core_ids=list(range(_B)))
    loss = _combine(res.results)
    return np.asarray(loss, dtype=np.float32)


# revision 13
# speedup vs baseline: 1.0811x; 1.0811x over previous
"""Trainium2 Bass kernel for debiased Sinkhorn divergence loss (geomloss-style).

Problem: B=8 batch of point clouds x,y [1024, 3]; loss = mean_b(
  (OT(x,y) - 0.5*OT(x,x) - 0.5*OT(y,y)) / N ), each OT via 17-step
log-domain Sinkhorn with geometric epsilon annealing.

Sharding: data-parallel over batch — each of the 8 NeuronCores runs one
batch element's three Sinkhorn problems; host combines the 24 OT values.

Device algorithm (per core), absorption form (validated == reference to
~1e-7 rel):
  g_new = g - eps*log( sum_i exp( (f_i + g_j - C_ij)/eps + log(1/N) ) )
  f_new = f - eps*log( sum_j exp( (g_j + f_i - C_ij)/eps + log(1/N) ) )
Cost matrices C (and C^T for the xy pair) are built on the PE from
host-prepared rank-5 factors.  Reductions always run along the SBUF free
dim: the per-partition potential enters as the ACT bias, the free-dim
potential is broadcast via PE rank-1 matmuls, the C term is fused in a
single DVE scalar_tensor_tensor pass, and exp+row-sum is one ACT pass
(accum_out).  The eps schedule is data-dependent (max over the batch of
each C stack) and is computed on host, entering as tiny input tables.
"""

import sys
import numpy as np

for _p in ("/opt/trn_rl_repo", "/root/.axon_site/_ro/trn_rl_repo"):
    if _p not in sys.path:
        sys.path.insert(0, _p)

_N = 1024          # points per cloud
_NT = 8            # 128-row tiles per matrix
_B = 8             # batch == cores
_NITER = 17        # 12 anneal + 5 extra
_EPS_FINAL = np.float32(0.05) ** np.float32(2.0)
_LOG_INV_N = float(-np.log(np.float32(_N)))

_cached = {}


def _build_program():
    import concourse.bass as bass
    import concourse.mybir as mybir
    from concourse import bacc, tile

    F32 = mybir.dt.float32
    AO = mybir.AluOpType
    AF = mybir.ActivationFunctionType

    # Patch the activation-table map so Exp and Ln resolve to the one set
    # that contains both ("natural_log_exp_and_others") — otherwise the
    # table-load pass alternates exp/ln sets every Sinkhorn half-step,
    # costing ~1.3us per ACT_TABLE_LOAD, ~260us total.
    import concourse.hw_specs as hw_specs
    import concourse.bacc as bacc_mod
    if not getattr(hw_specs.get_activation_tables, "_expln_patched", False):
        _orig_tables = hw_specs.get_activation_tables

        def _patched_tables(arch):
            tabs = dict(_orig_tables(arch))
            AFT = mybir.ActivationFunctionType
            combined = [n for n, s in tabs.items() if AFT.Exp in s and AFT.Ln in s]
            if combined:
                keep = combined[0]
                for n, s in list(tabs.items()):
                    if n != keep and (AFT.Exp in s or AFT.Ln in s):
                        tabs[n] = s - {AFT.Exp, AFT.Ln}
            return tabs

        _patched_tables._expln_patched = True
        hw_specs.get_activation_tables = _patched_tables
        bacc_mod.get_activation_tables = _patched_tables

    nc = bacc.Bacc("TRN2", target_bir_lowering=False, debug=False,
                   enable_asserts=False)

    def din(name, shape):
        return nc.dram_tensor(name, shape, F32, kind="ExternalInput").ap()

    # rank-5 cost factors: L* = [x0,x1,x2, 0.5*|x|^2, 1], R* = [-x0,-x1,-x2, 1, 0.5*|x|^2]
    Lx = din("Lx", [5, _N])
    Ly = din("Ly", [5, _N])
    Rx = din("Rx", [5, _N])
    Ry = din("Ry", [5, _N])
    ie = din("ie", [128, 3 * _NITER])    # 1/eps   per (grp,iter), col g*17+t
    nie = din("nie", [128, 3 * _NITER])  # -1/eps
    nep = din("nep", [128, 3 * _NITER])  # -eps
    ident = din("ident", [128, 128])     # identity for PE transpose
    out_d = nc.dram_tensor("out", [6, 128, _NT], F32, kind="ExternalOutput").ap()

    with tile.TileContext(nc) as tc:
        with (
            tc.tile_pool(name="cm", bufs=1) as cm_pool,
            tc.tile_pool(name="const", bufs=1) as const_pool,
            tc.tile_pool(name="state", bufs=2) as st_pool,
            tc.tile_pool(name="small", bufs=3) as sm_pool,
            tc.tile_pool(name="arg", bufs=3) as arg_pool,
            tc.tile_pool(name="escr", bufs=2) as e_pool,
        ):
            # ---- constants ----
            ident_sb = const_pool.tile([128, 128], F32, tag="ident")
            nc.sync.dma_start(ident_sb[:], ident[:])
            ie_sb = const_pool.tile([128, 3 * _NITER], F32, tag="ie")
            nie_sb = const_pool.tile([128, 3 * _NITER], F32, tag="nie")
            nep_sb = const_pool.tile([128, 3 * _NITER], F32, tag="nep")
            nc.sync.dma_start(ie_sb[:], ie[:])
            nc.sync.dma_start(nie_sb[:], nie[:])
            nc.sync.dma_start(nep_sb[:], nep[:])

            # ---- cost matrices ----
            # grp 0: xy needs C [i,j] and CT [j,i]; grp 1 (xx) / 2 (yy) are
            # symmetric so one matrix serves both update directions.
            with tc.tile_pool(name="fac", bufs=1) as fac_pool, \
                 tc.tile_pool(name="psC", bufs=4, space=bass.MemorySpace.PSUM) as ps_setup:
                facs = {}
                for nm, dr in (("Lx", Lx), ("Ly", Ly), ("Rx", Rx), ("Ry", Ry)):
                    ft = fac_pool.tile([5, _N], F32, tag=nm)
                    nc.sync.dma_start(ft[:], dr[:])
                    facs[nm] = ft

                cmats = {}
                specs = [("Cxy", "Lx", "Ry"), ("CTxy", "Ly", "Rx"),
                         ("Cxx", "Lx", "Rx"), ("Cyy", "Ly", "Ry")]
                k = 0
                for cname, lf, rf in specs:
                    ct = cm_pool.tile([128, _NT * _N], F32, tag=cname)
                    cmats[cname] = ct
                    for u in range(_NT):
                        for h in range(2):
                            ps = ps_setup.tile([128, 512], F32, tag="psC")
                            nc.tensor.matmul(
                                ps[:],
                                lhsT=facs[lf][:, u * 128:(u + 1) * 128],
                                rhs=facs[rf][:, h * 512:(h + 1) * 512],
                                start=True, stop=True,
                            )
                            dst = ct[:, u * _N + h * 512: u * _N + (h + 1) * 512]
                            if k % 2 == 0:
                                nc.vector.tensor_copy(dst, ps[:])
                            else:
                                nc.scalar.copy(dst, ps[:])
                            k += 1

            # matrices used by (g-update, f-update) per group
            mat_g = [cmats["CTxy"], cmats["Cxx"], cmats["Cyy"]]
            mat_f = [cmats["Cxy"], cmats["Cxx"], cmats["Cyy"]]

            with (
                tc.tile_pool(name="bc", bufs=3) as bc_pool,
                tc.tile_pool(name="psT", bufs=4, space=bass.MemorySpace.PSUM) as ps_tpose,
            ):
                # ---- initial potentials (zero) ----
                fcols = []
                gcols = []
                for g in range(3):
                    fz = st_pool.tile([128, _NT], F32, tag=f"fc{g}")
                    gz = st_pool.tile([128, _NT], F32, tag=f"gc{g}")
                    nc.vector.memset(fz[:], 0.0)
                    nc.vector.memset(gz[:], 0.0)
                    fcols.append(fz)
                    gcols.append(gz)

                def half_update(grp, t, cmat, bias_cols, bcast_cols, new_tag):
                    """One Sinkhorn half-step. Returns the new potential cols.

                    bias_cols: the potential being updated (enters ACT bias).
                    bcast_cols: the other potential (broadcast along free dim).
                    """
                    idx = grp * _NITER + t
                    # scale the broadcast-side potential by 1/eps, then move
                    # it to a true [1, N] row at partition 0: one PE column
                    # transpose per 128-block (engine APs must start at
                    # partition 0/32/64/96, so an [8,128] transpose is out).
                    sc = sm_pool.tile([128, _NT], F32, tag="sc")
                    nc.vector.tensor_scalar(
                        out=sc[:], in0=bcast_cols[:],
                        scalar1=ie_sb[:, idx:idx + 1], scalar2=None, op0=AO.mult)
                    rowv = sm_pool.tile([1, _N], F32, tag="rowv")
                    for h in range(2):
                        tp = ps_tpose.tile([1, 512], F32, tag="tp")
                        for q in range(4):
                            u = h * 4 + q
                            nc.tensor.transpose(
                                tp[0:1, q * 128:(q + 1) * 128],
                                sc[:, u:u + 1], ident_sb[:])
                        nc.vector.tensor_copy(
                            rowv[0:1, h * 512:(h + 1) * 512], tp[:])
                    # broadcast each 128-slice across all partitions on the
                    # (otherwise idle) GPSIMD engine
                    r1 = bc_pool.tile([128, _N], F32, tag="bc")
                    for u in range(_NT):
                        nc.gpsimd.partition_broadcast(
                            r1[:, u * 128:(u + 1) * 128],
                            rowv[0:1, u * 128:(u + 1) * 128])
                    # ACT bias: bias_cols/eps + log(1/N)
                    bias = sm_pool.tile([128, _NT], F32, tag="bias")
                    nc.vector.tensor_scalar(
                        out=bias[:], in0=bias_cols[:],
                        scalar1=ie_sb[:, idx:idx + 1], scalar2=_LOG_INV_N,
                        op0=AO.mult, op1=AO.add)
                    S = sm_pool.tile([128, _NT], F32, tag="S")
                    for u in range(_NT):
                        argt = arg_pool.tile([128, _N], F32, tag="arg")
                        nc.vector.scalar_tensor_tensor(
                            out=argt[:],
                            in0=cmat[:, u * _N:(u + 1) * _N],
                            scalar=nie_sb[:, idx:idx + 1],
                            in1=r1[:],
                            op0=AO.mult, op1=AO.add)
                        et = e_pool.tile([128, _N], F32, tag="E")
                        nc.scalar.activation(
                            et[:], argt[:], AF.Exp,
                            bias=bias[:, u:u + 1], scale=1.0,
                            accum_out=S[:, u:u + 1])
                    logS = sm_pool.tile([128, _NT], F32, tag="logS")
                    nc.scalar.activation(logS[:], S[:], AF.Ln)
                    new_cols = st_pool.tile([128, _NT], F32, tag=new_tag)
                    nc.vector.scalar_tensor_tensor(
                        out=new_cols[:], in0=logS[:],
                        scalar=nep_sb[:, idx:idx + 1], in1=bias_cols[:],
                        op0=AO.mult, op1=AO.add)
                    return new_cols

                for t in range(_NITER):
                    for g in range(3):
                        gcols[g] = half_update(
                            g, t, mat_g[g], gcols[g], fcols[g], f"gc{g}")
                        fcols[g] = half_update(
                            g, t, mat_f[g], fcols[g], gcols[g], f"fc{g}")

                for g in range(3):
                    nc.sync.dma_start(out_d[2 * g], fcols[g][:, :])
                    nc.sync.dma_start(out_d[2 * g + 1], gcols[g][:, :])

    nc.compile()
    return nc


def _get_program():
    if "nc" not in _cached:
        _cached["nc"] = _build_program()
    return _cached["nc"]


def _host_prep(template, source):
    """Per-core input tensors + shared eps tables (computed from batch max)."""
    template = np.asarray(template, np.float32)
    source = np.asarray(source, np.float32)
    onev = np.ones(_N, np.float32)

    def lfac(x):
        x2 = (x * x).sum(-1).astype(np.float32)
        return np.ascontiguousarray(
            np.stack([x[:, 0], x[:, 1], x[:, 2],
                      np.float32(0.5) * x2, onev]))

    def rfac(x):
        x2 = (x * x).sum(-1).astype(np.float32)
        return np.ascontiguousarray(
            np.stack([-x[:, 0], -x[:, 1], -x[:, 2],
                      onev, np.float32(0.5) * x2]))

    def cost_max(x, y):
        # fp32 like the reference; only the batch max is consumed
        x2 = (x * x).sum(-1)
        y2 = (y * y).sum(-1)
        xy = np.einsum("bnd,bmd->bnm", x, y, dtype=np.float32)
        c = np.float32(0.5) * (x2[:, :, None] + y2[:, None, :] - 2.0 * xy)
        return np.float32(c.max())

    scheds = []
    for cmax in (cost_max(template, source),
                 cost_max(template, template),
                 cost_max(source, source)):
        eps_start = np.maximum(cmax, np.float32(2.0) * _EPS_FINAL)
        t = np.arange(12, dtype=np.float32) / np.float32(11.0)
        sch = (eps_start * (_EPS_FINAL / eps_start) ** t).astype(np.float32)
        scheds.append(np.concatenate(
            [sch, np.full(5, _EPS_FINAL, np.float32)]))
    eps = np.concatenate(scheds)                       # [51]
    ie = np.broadcast_to(np.float32(1.0) / eps, (128, 51)).copy()
    nie = np.broadcast_to(np.float32(-1.0) / eps, (128, 51)).copy()
    nep = np.broadcast_to(-eps, (128, 51)).copy()
    ident = np.eye(128, dtype=np.float32)

    in_maps = []
    for b in range(_B):
        x, y = template[b], source[b]
        in_maps.append({
            "Lx": lfac(x), "Ly": lfac(y),
            "Rx": rfac(x), "Ry": rfac(y),
            "ie": ie, "nie": nie, "nep": nep, "ident": ident,
        })
    return in_maps, eps


def _combine(results):
    """results: per-core dict with 'out' [6,128,8] -> scalar loss."""
    ots = np.zeros((3, _B), np.float32)
    for b, res in enumerate(results):
        o = np.asarray(res["out"], np.float32)
        for g in range(3):
            ots[g, b] = o[2 * g].mean(dtype=np.float32) + \
                o[2 * g + 1].mean(dtype=np.float32)
    div = ots[0] - np.float32(0.5) * (ots[1] + ots[2])
    return np.float32((div / np.float32(_N)).mean(dtype=np.float32))


def kernel(template, source):
    from concourse.bass_utils import run_bass_kernel_spmd

    nc = _get_program()
    in_maps, _ = _host_prep(template, source)
    res = run_bass_kernel_spmd(nc, in_maps, core_ids=list(range(_B)))
    loss = _combine(res.results)
    return np.asarray(loss, dtype=np.float32)
